# revision 17
# baseline (speedup 1.0000x reference)
"""GQA decode attention (B=32, q_len=1, T=4096, 32 q heads / 8 kv heads, hd=128)
on 8 Trainium2 NeuronCores.

Sharding: tensor-parallel over kv heads — core h owns kv head h (4 q heads),
its slice of wq/wk/wv (ColumnParallel) and wo (RowParallel), and the
cache_k/cache_v slices for that head. Each core computes a partial [B, DIM]
output (RowParallel wo); the host sums the 8 partials.

This is the memory-bound regime: the dominant HBM traffic is the KV cache.
The correctness gate is rel_err < 2e-2, far looser than what fp32 KV needs,
so the cache is read in pure bf16 (half the bytes of the fp32/hi-lo builds):
  - K cache bf16 [B, 128, T]; scores = Kh.qh + Kh.ql with q split into
    bf16 hi+lo on-chip (q keeps near-fp32 precision; K rounding ~1e-3 rel
    error on the output, well within tolerance).
  - V cache bf16 with a ones column appended per 128-key chunk so the PV
    matmul also emits the softmax denominator.
  - wqkv / wo in bf16 (their rounding contributes ~1e-3).
  - KV for GROUP consecutive batches is packed side by side in DRAM so each
    dma_start moves GROUP MiB contiguously (better DMA efficiency).
Host-side algebraic prep (folded into the weights, so the device kernel is
pure matmul + softmax): RoPE is a fixed linear map at q_len==1 and is folded
into wq/wk; the 1/sqrt(head_dim) scale is folded into wq; weights are
pre-transposed and the kv cache pre-permuted into tensor-engine layouts.
"""

import numpy as np

B = 32
DIM = 4096
HD = 128
NKV = 8
NG = 4          # q heads per kv head
T = 4096
NT = 32         # T / 128 key tiles
ND = 32         # DIM / 128 contraction chunks
N_CORES = 8
VW = 129        # V tile width: 128 value dims + 1 ones column
GROUP = 2       # batches packed per KV dma

_PROG_CACHE = {}


def _build_program():
    import concourse.mybir as mybir
    import concourse.tile as tile
    from concourse import bacc

    fp32 = mybir.dt.float32
    bf16 = mybir.dt.bfloat16
    af = mybir.ActivationFunctionType

    nc = bacc.Bacc("TRN2", target_bir_lowering=False, debug=False,
                   num_devices=N_CORES)

    xTp_d = nc.dram_tensor("xTp", [128, 2 * ND * B], bf16, kind="ExternalInput").ap()
    # wqkv packed partition-major: [p, (chunk, 768)] so the load is one DMA
    # with 48KB-contiguous per-partition descriptors
    wqkvT_d = nc.dram_tensor("wqkvT", [128, ND * 768], bf16, kind="ExternalInput").ap()
    woT_d = nc.dram_tensor("woT", [NG * HD, DIM], bf16, kind="ExternalInput").ap()
    # K/V for GROUP batches packed side by side: one contiguous dma per group
    KTg_d = nc.dram_tensor("KTg", [B // GROUP, 128, GROUP * T], bf16,
                           kind="ExternalInput").ap()
    Vpg_d = nc.dram_tensor("Vpg", [B // GROUP, 128, GROUP * NT * VW], bf16,
                           kind="ExternalInput").ap()
    ident_d = nc.dram_tensor("ident", [128, 128], fp32, kind="ExternalInput").ap()
    out_d = nc.dram_tensor("out", [B, DIM], bf16, kind="ExternalOutput").ap()

    with tile.TileContext(nc) as tc:
        from contextlib import ExitStack
        with ExitStack() as ctx:
            const_pool = ctx.enter_context(tc.tile_pool(name="const", bufs=1))
            kv_pool = ctx.enter_context(tc.tile_pool(name="kv", bufs=3))
            pr_pool = ctx.enter_context(tc.tile_pool(name="pr", bufs=3))
            small = ctx.enter_context(tc.tile_pool(name="small", bufs=2))

            # startup loads stay OFF the sync queue so the K-cache stream owns
            # it from t=0: ident/xTp ride the scalar ring, wqkv rides the
            # gpsimd ring ahead of the V stream (2 chunks so projections can
            # start at the halfway point)
            ident_sb = const_pool.tile([128, 128], fp32, name="ident_sb")
            nc.scalar.dma_start(ident_sb[:], ident_d[:])
            xTp_sb = const_pool.tile([128, 2 * ND * B], bf16, name="xTp_sb")
            nc.scalar.dma_start(xTp_sb[:], xTp_d[:])
            wqkv_sb = const_pool.tile([128, ND * 768], bf16, name="wqkv_sb")
            WH = ND * 768 // 2
            nc.gpsimd.dma_start(wqkv_sb[:, 0:WH], wqkvT_d[:, 0:WH])
            nc.gpsimd.dma_start(wqkv_sb[:, WH:2 * WH], wqkvT_d[:, WH:2 * WH])

            woT_sb = [const_pool.tile([128, DIM], bf16, name=f"woT{g}_sb",
                                      tag=f"woT{g}") for g in range(NG)]

            # ---- QKV projections: qT[o,b], kT[o,b], v[b,o] ----
            # q hi/lo packed in one tile: col (s, g, b), s=0 hi / s=1 lo
            qT2_sb = const_pool.tile([128, 2 * NG * B], bf16, name="qT2_sb")
            kT_sb = const_pool.tile([128, B], bf16, name="kT_sb")
            v_sb = const_pool.tile([B, VW], bf16, name="v_sb")

            # projection PSUM: its own scope, released before attention pools
            with tc.tile_pool(name="ppsum", bufs=1, space="PSUM") as ppsum:
                psq = [ppsum.tile([128, B], fp32, name=f"psq{g}", tag=f"psq{g}")
                       for g in range(NG)]
                psk = ppsum.tile([128, B], fp32, name="psk", tag="psk")
                psv = ppsum.tile([B, HD], fp32, name="psv", tag="psv")
                HB = ND * B   # offset of the lo half in xTp
                for n in range(ND):
                    wch = wqkv_sb[:, 768 * n:768 * (n + 1)]
                    xh = xTp_sb[:, B * n:B * (n + 1)]
                    xl = xTp_sb[:, HB + B * n:HB + B * (n + 1)]
                    st, sp = (n == 0), (n == ND - 1)
                    # x hi/lo split against bf16 weights: W.x ~= Wh.xh + Wh.xl
                    for g in range(NG):
                        wh = wch[:, 128 * g:128 * (g + 1)]
                        nc.tensor.matmul(psq[g][:], wh, xh, start=st, stop=False)
                        nc.tensor.matmul(psq[g][:], wh, xl, start=False, stop=sp)
                    nc.tensor.matmul(psk[:], wch[:, 512:640], xh, start=st, stop=False)
                    nc.tensor.matmul(psk[:], wch[:, 512:640], xl, start=False, stop=sp)
                    nc.tensor.matmul(psv[:], xh, wch[:, 640:768], start=st, stop=False)
                    nc.tensor.matmul(psv[:], xl, wch[:, 640:768], start=False, stop=sp)
                GB = NG * B   # offset of the lo half in qT2
                for g in range(NG):
                    nc.vector.tensor_copy(qT2_sb[:, B * g:B * (g + 1)], psq[g][:])
                    nc.vector.tensor_sub(qT2_sb[:, GB + B * g:GB + B * (g + 1)],
                                         psq[g][:],
                                         qT2_sb[:, B * g:B * (g + 1)])
                nc.vector.tensor_copy(kT_sb[:], psk[:])
                nc.vector.tensor_copy(v_sb[:, 0:HD], psv[:])
                nc.vector.memset(v_sb[:, HD:VW], 1.0)

            spsum = ctx.enter_context(tc.tile_pool(name="spsum", bufs=3, space="PSUM"))
            opsum = ctx.enter_context(tc.tile_pool(name="opsum", bufs=3, space="PSUM"))
            wpsum = ctx.enter_context(tc.tile_pool(name="wpsum", bufs=2, space="PSUM"))

            # q view with free index (s, g, b) -> [p, b, (s g)]: one [128, 8]
            # moving operand per batch carrying q-hi and q-lo side by side
            qT2_re = qT2_sb.rearrange("p (s g b) -> p b s g", s=2, b=B)
            attnT_sb = const_pool.tile([128, NG * B], bf16, name="attnT_sb")
            attnT_re = attnT_sb.rearrange("p (g b) -> p b g", b=B)

            # ---- attention, software-pipelined: scores(b) interleaved with
            # PV(b-1) tile-by-tile so the PV matmuls (probs-stationary, tiny
            # weight load) hide inside the K-tile LDWEIGHTS shadow and the PE
            # never idles ----
            Kg_cur = Vg_cur = None
            prev = None          # (V view, probs, psO) of batch b-1

            def tail(b_prev, psO_prev):
                recip = small.tile([NG, 1], fp32, name="recip", tag="recip")
                nc.vector.reciprocal(recip[:], psO_prev[0:NG, HD:VW])
                attn_b = small.tile([NG, HD], fp32, name="attn_b", tag="attn_b")
                nc.vector.tensor_scalar_mul(attn_b[:], psO_prev[0:NG, 0:HD],
                                            recip[:])
                nc.tensor.transpose(psO_prev[:, VW:VW + NG], attn_b[:],
                                    ident_sb[0:NG, 0:NG])
                nc.vector.tensor_copy(attnT_re[:, b_prev],
                                      psO_prev[:, VW:VW + NG])

            # output projection, issued in two batch-halves: half 0 overlaps
            # the second half of the attention phase
            out_h = [const_pool.tile([B // 2, DIM], bf16, name=f"out{h}_sb",
                                     tag=f"out{h}") for h in range(2)]
            HB2 = B // 2

            def wo_half(h):
                for jj in range(DIM // 512):
                    psW = wpsum.tile([HB2, 512], fp32, name="psW", tag="psW")
                    for g in range(NG):
                        nc.tensor.matmul(
                            psW[:],
                            attnT_sb[:, B * g + HB2 * h:B * g + HB2 * (h + 1)],
                            woT_sb[g][:, 512 * jj:512 * (jj + 1)],
                            start=(g == 0), stop=(g == NG - 1))
                    nc.vector.tensor_copy(out_h[h][:, 512 * jj:512 * (jj + 1)],
                                          psW[:])

            for b in range(B):
                j, bb = divmod(b, GROUP)
                if bb == 0:
                    if b == 8:
                        # load the output-projection weights once the initial
                        # burst is over but well before the mid-kernel wo half
                        for g in range(NG):
                            nc.sync.dma_start(woT_sb[g][:],
                                              woT_d[128 * g:128 * (g + 1), :])
                if b == 20:
                    # attnT for batches 0..15 is complete (tail(15) ran at
                    # b==16); fold in the first wo half while the cache
                    # stream still owns the DMA queues
                    wo_half(0)
                if bb == 0:
                    Kg_cur = kv_pool.tile([128, GROUP * T], bf16, name="Kg_sb",
                                          tag="K")
                    nc.sync.dma_start(Kg_cur[:], KTg_d[j])
                    # V rides the gpsimd (SWDGE) queue: separate issuing engine
                    # and separate SDMA ring from the K stream
                    Vg_cur = kv_pool.tile([128, GROUP * NT * VW], bf16,
                                          name="Vg_sb", tag="V")
                    nc.gpsimd.dma_start(Vg_cur[:], Vpg_d[j])
                    for b2 in range(GROUP):
                        # new-token key/value for every batch of the group:
                        # overwrite cache column t=4095 (key) and the t=4095
                        # V row (partition 127 of the last chunk; cross-
                        # partition move -> tiny DMA on the scalar ring)
                        nc.vector.tensor_copy(
                            Kg_cur[:, b2 * T + T - 1:b2 * T + T],
                            kT_sb[:, j * GROUP + b2:j * GROUP + b2 + 1])
                        off = b2 * NT * VW + VW * (NT - 1)
                        nc.scalar.dma_start(
                            Vg_cur[127:128, off:off + HD],
                            v_sb[j * GROUP + b2:j * GROUP + b2 + 1, 0:HD])

                K_sb = Kg_cur[:, bb * T:(bb + 1) * T]
                V_sb = Vg_cur[:, bb * NT * VW:(bb + 1) * NT * VW]

                q8 = qT2_re[:, b]  # [128, 2, 4] strided: (qh | ql) x 4 heads
                # psS col layout (n, s, g): the hi and lo partial scores land
                # side by side and are summed by one strided DVE add below
                psS = spsum.tile([128, 2 * NG * NT], fp32, name="psS", tag="psS")
                psO_prev = None
                if prev is not None:
                    psO_prev = opsum.tile([128, VW + NG], fp32, name="psO",
                                          tag="psO")
                for n in range(NT):
                    # K bf16; q kept near-fp32 via on-chip hi/lo split:
                    # one 8-column matmul computes K.qh and K.ql together
                    sl = psS[:, 2 * NG * n:2 * NG * (n + 1)]
                    kh = K_sb[:, 128 * n:128 * (n + 1)]
                    nc.tensor.matmul(sl, kh, q8, start=True, stop=True)
                    if prev is not None:
                        pV, pprobs = prev
                        nc.tensor.matmul(psO_prev[0:NG, 0:VW],
                                         pprobs[:, NG * n:NG * (n + 1)],
                                         pV[:, VW * n:VW * (n + 1)],
                                         start=(n == 0), stop=(n == NT - 1),
                                         skip_group_check=True)
                # exp(hi+lo) = exp(hi)*exp(lo): ACT exponentiates the whole
                # interleaved psS (DVE cannot read two PSUM operands), then a
                # strided bf16 DVE multiply folds the hi/lo pairs into probs
                e2 = pr_pool.tile([128, 2 * NG * NT], bf16, name="e2", tag="e2")
                for c in range(2):
                    cw = NG * NT
                    nc.scalar.activation(e2[:, cw * c:cw * (c + 1)],
                                         psS[:, cw * c:cw * (c + 1)], af.Exp)
                probs = pr_pool.tile([128, NG * NT], bf16, name="probs",
                                     tag="probs")
                e2_re = e2.rearrange("p (n s g) -> p s n g", s=2, g=NG)
                probs_re = probs.rearrange("p (n g) -> p n g", g=NG)
                for c in range(4):
                    cn = NT // 4
                    nc.vector.tensor_mul(probs_re[:, cn * c:cn * (c + 1)],
                                         e2_re[:, 0, cn * c:cn * (c + 1)],
                                         e2_re[:, 1, cn * c:cn * (c + 1)])
                if prev is not None:
                    tail(b - 1, psO_prev)
                prev = (V_sb, probs)

            # epilogue: PV + tail for the last batch
            pV, pprobs = prev
            psO_last = opsum.tile([128, VW + NG], fp32, name="psO", tag="psO")
            for n in range(NT):
                nc.tensor.matmul(psO_last[0:NG, 0:VW],
                                 pprobs[:, NG * n:NG * (n + 1)],
                                 pV[:, VW * n:VW * (n + 1)],
                                 start=(n == 0), stop=(n == NT - 1))
            tail(B - 1, psO_last)

            # ---- second wo half + store ----
            wo_half(1)
            nc.sync.dma_start(out_d[0:HB2, :], out_h[0][:])
            nc.sync.dma_start(out_d[HB2:B, :], out_h[1][:])

    nc.compile()
    return nc


def _get_program():
    key = "nc_bf16"
    if key not in _PROG_CACHE:
        _PROG_CACHE[key] = _build_program()
    return _PROG_CACHE[key]


def _host_prep(x, freqs_cos, freqs_sin, cache_k, cache_v, wq, wk, wv, wo):
    """Build the 8 per-core input maps."""
    import ml_dtypes
    bfl = ml_dtypes.bfloat16
    f32 = np.float32
    x = np.asarray(x, f32)
    cos = np.asarray(freqs_cos, f32).reshape(-1)[:HD // 2]
    sin = np.asarray(freqs_sin, f32).reshape(-1)[:HD // 2]
    wq = np.asarray(wq, f32)
    wk = np.asarray(wk, f32)
    wv = np.asarray(wv, f32)
    wo = np.asarray(wo, f32)
    cache_k = np.asarray(cache_k, f32)
    cache_v = np.asarray(cache_v, f32)

    def rope_fold(w, nheads):
        w4 = w.reshape(nheads, HD // 2, 2, DIM)
        a, bb = w4[:, :, 0, :], w4[:, :, 1, :]
        c = cos[None, :, None]
        s = sin[None, :, None]
        out = np.empty_like(w4)
        out[:, :, 0, :] = a * c - bb * s
        out[:, :, 1, :] = a * s + bb * c
        return out.reshape(nheads * HD, DIM)

    wq_r = rope_fold(wq, NKV * NG) * f32(1.0 / np.sqrt(HD))
    wk_r = rope_fold(wk, NKV)

    x2 = x.reshape(B, DIM)
    xTp = np.ascontiguousarray(
        x2.T.reshape(ND, 128, B).transpose(1, 0, 2)).reshape(128, ND * B)
    xh = xTp.astype(bfl)
    xTp = np.concatenate([xh, (xTp - xh.astype(f32)).astype(bfl)], axis=1)

    # K: [h, b, d, t] bf16, GROUP batches side by side
    KT_all = np.ascontiguousarray(
        cache_k.transpose(2, 0, 3, 1).astype(bfl))         # [h, B, 128, T]
    KTg_all = KT_all.reshape(NKV, B // GROUP, GROUP, 128, T).transpose(
        0, 1, 3, 2, 4).reshape(NKV, B // GROUP, 128, GROUP * T)
    KTg_all = np.ascontiguousarray(KTg_all)

    # V: [h, b, p, n, d] + ones column per chunk, bf16, GROUP side by side
    cv = cache_v.reshape(B, NT, 128, NKV, HD)
    Vp_all = np.ones((NKV, B, 128, NT, VW), bfl)
    Vp_all[..., :HD] = cv.transpose(3, 0, 2, 1, 4).astype(bfl)
    Vpg_all = Vp_all.reshape(NKV, B // GROUP, GROUP, 128, NT * VW).transpose(
        0, 1, 3, 2, 4).reshape(NKV, B // GROUP, 128, GROUP * NT * VW)
    Vpg_all = np.ascontiguousarray(Vpg_all)

    ident = np.eye(128, dtype=f32)

    in_maps = []
    for h in range(N_CORES):
        wqkvT = np.concatenate([
            wq_r[h * NG * HD:(h + 1) * NG * HD],
            wk_r[h * HD:(h + 1) * HD],
            wv[h * HD:(h + 1) * HD],
        ], axis=0).T.astype(bfl)                            # [4096, 768]
        # partition-major packing: [p, (chunk, 768)]
        wqkvT = np.ascontiguousarray(
            wqkvT.reshape(ND, 128, 768).transpose(1, 0, 2).reshape(128, ND * 768))
        woT = np.ascontiguousarray(
            wo[:, h * NG * HD:(h + 1) * NG * HD].T.astype(bfl))
        m = {
            "xTp": xTp,
            "wqkvT": wqkvT,
            "woT": woT,
            "KTg": KTg_all[h],
            "Vpg": Vpg_all[h],
            "ident": ident,
        }
        in_maps.append(m)
    return in_maps


def _kernel_numpy_fallback(x, start_pos, freqs_cos, freqs_sin, cache_k, cache_v,
                           wq, wk, wv, wo):
    """Reference-equivalent numpy path for shapes this kernel isn't built for."""
    f32 = np.float32
    start_pos = int(start_pos)
    x = np.asarray(x, f32)
    bsz, seqlen, _ = x.shape
    n_rep = 4
    hd = HD

    def rope(t, c, s):
        tr = t.reshape(*t.shape[:-1], hd // 2, 2)
        a, b2 = tr[..., 0], tr[..., 1]
        c = c[None, :, None, :]
        s = s[None, :, None, :]
        out = np.stack([a * c - b2 * s, a * s + b2 * c], axis=-1)
        return out.reshape(t.shape)

    xq = (x @ np.asarray(wq, f32).T).reshape(bsz, seqlen, NKV * n_rep, hd)
    xk = (x @ np.asarray(wk, f32).T).reshape(bsz, seqlen, NKV, hd)
    xv = (x @ np.asarray(wv, f32).T).reshape(bsz, seqlen, NKV, hd)
    fc = np.asarray(freqs_cos, f32)
    fs = np.asarray(freqs_sin, f32)
    xq = rope(xq, fc, fs)
    xk = rope(xk, fc, fs)
    ck = np.array(cache_k, f32, copy=True)
    cvv = np.array(cache_v, f32, copy=True)
    ck[:, start_pos:start_pos + seqlen] = xk
    cvv[:, start_pos:start_pos + seqlen] = xv
    keys = ck[:, :start_pos + seqlen]
    values = cvv[:, :start_pos + seqlen]
    q = xq.reshape(bsz, seqlen, NKV, n_rep, hd)
    scale = 1.0 / np.sqrt(hd)
    scores = np.einsum('bsgrd,btgd->bgrst', q, keys) * scale
    scores = scores - scores.max(axis=-1, keepdims=True)
    e = np.exp(scores)
    probs = e / e.sum(axis=-1, keepdims=True)
    out = np.einsum('bgrst,btgd->bsgrd', probs, values)
    out = out.reshape(bsz, seqlen, NKV * n_rep * hd)
    return (out @ np.asarray(wo, f32).T).astype(f32)


TRACE = False          # set True (e.g. from test.py) to neuron-profile the run
TRACE_KWARGS = {}
LAST_RESULT = None     # BassKernelResults of the most recent device run


def kernel(x, start_pos, freqs_cos, freqs_sin, cache_k, cache_v, wq, wk, wv, wo):
    global LAST_RESULT
    x = np.asarray(x)
    if (int(start_pos) != T - 1 or x.shape != (B, 1, DIM)
            or np.asarray(cache_k).shape != (B, T, NKV, HD)):
        return _kernel_numpy_fallback(x, start_pos, freqs_cos, freqs_sin,
                                      cache_k, cache_v, wq, wk, wv, wo)

    from concourse.bass_utils import run_bass_kernel_spmd

    nc = _get_program()
    in_maps = _host_prep(x, freqs_cos, freqs_sin, cache_k, cache_v,
                         wq, wk, wv, wo)
    res = run_bass_kernel_spmd(nc, in_maps, list(range(N_CORES)),
                               trace=TRACE, **TRACE_KWARGS)
    LAST_RESULT = res
    out = np.zeros((B, DIM), np.float64)
    for i in range(N_CORES):
        out += res.results[i]["out"]
    return out.astype(np.float32).reshape(B, 1, DIM)
